# revision 1
# baseline (speedup 1.0000x reference)
"""Bass/Tile Trainium2 kernel for the attention-LSTM greedy decoder.

Sharding (8 cores, one TRN2 chip):
  - vocab-sharded output projection: core c owns Fw rows [4096c, 4096(c+1))
    (core 7: 3328 real rows + 768 zero pads)
  - batch-sharded attention: core c owns batches [8c, 8c+8) for the
    tanh-attention elementwise work + context matvecs
  - LSTM + state replicated on every core (full batch of 64)
  - one AllGather per step carries (ctxT of step t, argmax-candidates of
    step t-1); greedy feedback resolved identically on every core, next
    input embedding fetched by indirect DMA from a replicated DRAM copy
    of emb_table.

Layout: "transposed world" — state h/c, xin, logits-lhs all stored [feat, batch]
so every matmul's stationary operand is ready without per-step transposes.
Only w (softmax weights), ctx rows and the gathered embedding get transposed,
via PE transpose-mode.

dtype: fp32 everywhere by default.  cfg['proj'] selects the projection matmul
flavor: 'f32' (exact) or 'bf16x3' (hi/lo split, ~1.5e-5 rel err, ~1.4x faster).
"""

import sys

sys.path.insert(0, "/opt/trn_rl_repo")

import numpy as np
import concourse.bass as bass
import concourse.bacc as bacc
import concourse.tile as tile
from concourse import mybir
from concourse import bass_utils

F32 = mybir.dt.float32
BF16 = mybir.dt.bfloat16
U32 = mybir.dt.uint32
AF = mybir.ActivationFunctionType
ALU = mybir.AluOpType

CORES = 8
B = 64          # batch
BL = B // CORES  # local batch (8)
S = 128         # encoder length
H = 256         # hidden
D2 = 512        # 2H
D3 = 768        # 3H
G4 = 1024       # 4H
VSH = 4096      # vocab shard per core (padded)
V = 32000


def build(T: int, proj_mode: str, fb_nonzero: bool):
    nc = bacc.Bacc("TRN2", target_bir_lowering=False, debug=False,
                   num_devices=CORES)

    # ---------------- DRAM I/O ----------------
    def din(name, shape, dt=F32):
        return nc.dram_tensor(name, shape, dt, kind="ExternalInput")

    i_encT = din("encT", [S, BL, D2])            # enc[s, b_local, d]
    i_epT = din("epT", [128, 4, BL, S])          # tanh-input constant, transposed
    i_h0T = din("h0T", [2, H, B])
    i_c0T = din("c0T", [2, H, B])
    i_W2T = din("W2T", [H, D2])
    i_VwT = din("VwT", [128, 4])
    i_WihT_f = din("WihT_f", [D3, G4])
    i_WihT_b = din("WihT_b", [D3, G4])
    i_WhhT_f = din("WhhT_f", [H, G4])
    i_WhhT_b = din("WhhT_b", [H, G4])
    i_bsc_f = din("bsc_f", [128, 8])
    i_bsc_b = din("bsc_b", [128, 8])
    i_emb = din("emb", [CORES * VSH, H])         # replicated full (padded) table
    i_embT0 = din("embT0", [128, 2, B])          # emb_table[0] broadcast, chunked
    i_sel = din("sel", [B, BL])                  # batch-selection matrix (0/1)
    i_vbase = din("vbase", [B, 1])               # 4096*c in every row
    i_ident = din("ident", [128, 128])
    if proj_mode == "f32":
        i_FwT = din("FwT", [D2, VSH])
    else:
        i_FwT_hi = din("FwT_hi", [D2, VSH], BF16)
        i_FwT_lo = din("FwT_lo", [D2, VSH], BF16)
    if fb_nonzero:
        i_FbB = din("FbB", [B, VSH])             # Fb broadcast along batch

    o_ys = nc.dram_tensor("ys_out", [T, B, VSH], F32, kind="ExternalOutput")
    o_h = nc.dram_tensor("hT_out", [2, H, B], F32, kind="ExternalOutput")
    o_c = nc.dram_tensor("cT_out", [2, H, B], F32, kind="ExternalOutput")

    with tile.TileContext(nc) as tc:
        with (
            tc.tile_pool(name="consts", bufs=1) as consts,
            tc.tile_pool(name="state", bufs=1) as state,
            tc.tile_pool(name="work", bufs=2) as work,
            tc.tile_pool(name="attn", bufs=2) as attn,
            tc.tile_pool(name="ps_big", bufs=2, space="PSUM") as ps_big,
            tc.tile_pool(name="ps_lstm", bufs=3, space="PSUM") as ps_lstm,
            tc.tile_pool(name="ps_strip", bufs=3, space="PSUM") as ps_strip,
            tc.tile_pool(name="dram", bufs=2, space="DRAM") as dram,
        ):
            # ---------------- load constants ----------------
            enc_sb = consts.tile([S, BL, D2], F32)      # [s, b, d] 16KB/p
            nc.sync.dma_start(out=enc_sb[:], in_=i_encT[:, :, :])
            epT_sb = consts.tile([128, 4, BL, S], F32)  # 16KB/p
            nc.sync.dma_start(out=epT_sb[:], in_=i_epT[:, :, :, :])
            W2T_sb = consts.tile([128, 2, D2], F32)
            nc.sync.dma_start(out=W2T_sb[:], in_=i_W2T.ap().rearrange("(k p) d -> p k d", p=128))
            VwT_sb = consts.tile([128, 4], F32)
            nc.sync.dma_start(out=VwT_sb[:], in_=i_VwT[:, :])
            WihT = {}
            WhhT = {}
            bsc = {}
            for d, iW, iU, ib in (("f", i_WihT_f, i_WhhT_f, i_bsc_f),
                                  ("b", i_WihT_b, i_WhhT_b, i_bsc_b)):
                WihT[d] = consts.tile([128, 6, G4], F32, tag=f"WihT_{d}", name=f"WihT_{d}")
                nc.sync.dma_start(out=WihT[d][:], in_=iW.ap().rearrange("(k p) m -> p k m", p=128))
                WhhT[d] = consts.tile([128, 2, G4], F32, tag=f"WhhT_{d}", name=f"WhhT_{d}")
                nc.sync.dma_start(out=WhhT[d][:], in_=iU.ap().rearrange("(k p) m -> p k m", p=128))
                bsc[d] = consts.tile([128, 8], F32, tag=f"bsc_{d}", name=f"bsc_{d}")
                nc.sync.dma_start(out=bsc[d][:], in_=ib[:, :])
            if proj_mode == "f32":
                FwT_sb = consts.tile([128, 4, VSH], F32)
                nc.sync.dma_start(out=FwT_sb[:], in_=i_FwT.ap().rearrange("(k p) v -> p k v", p=128))
            else:
                FwT_hi_sb = consts.tile([128, 4, VSH], BF16)
                nc.sync.dma_start(out=FwT_hi_sb[:], in_=i_FwT_hi.ap().rearrange("(k p) v -> p k v", p=128))
                FwT_lo_sb = consts.tile([128, 4, VSH], BF16)
                nc.sync.dma_start(out=FwT_lo_sb[:], in_=i_FwT_lo.ap().rearrange("(k p) v -> p k v", p=128))
            sel_sb = consts.tile([B, BL], F32)
            nc.sync.dma_start(out=sel_sb[:], in_=i_sel[:, :])
            vbase_sb = consts.tile([B, 1], F32)
            nc.sync.dma_start(out=vbase_sb[:], in_=i_vbase[:, :])
            ident_sb = consts.tile([128, 128], F32)
            nc.sync.dma_start(out=ident_sb[:], in_=i_ident[:, :])
            if fb_nonzero:
                FbB_sb = consts.tile([B, VSH], F32)
                nc.sync.dma_start(out=FbB_sb[:], in_=i_FbB[:, :])

            # ---------------- state ----------------
            hT = {"f": state.tile([128, 2, B], F32, tag="hT_f", name="hT_f"),
                  "b": state.tile([128, 2, B], F32, tag="hT_b", name="hT_b")}
            cT = {"f": state.tile([128, 2, B], F32, tag="cT_f", name="cT_f"),
                  "b": state.tile([128, 2, B], F32, tag="cT_b", name="cT_b")}
            for di, d in enumerate(("f", "b")):
                nc.sync.dma_start(out=hT[d][:], in_=i_h0T[di].rearrange("(k p) b -> p k b", p=128))
                nc.sync.dma_start(out=cT[d][:], in_=i_c0T[di].rearrange("(k p) b -> p k b", p=128))
            xinT = state.tile([128, 6, B], F32, tag="xinT")   # 0-3 ctxT, 4-5 embT
            nc.sync.dma_start(out=xinT[:, 4:6, :], in_=i_embT0[:, :, :])
            zero_cand = consts.tile([B, 2], F32)
            nc.vector.memset(zero_cand[:], 0.0)

            prev_cand = None  # sbuf [B, 2] tile from previous step's projection

            for t in range(T):
                # ====== attention (from current h state) ======
                # u = h_f @ W2.T  -> [B, D2] (full batch, batch-on-partition)
                ps_u = ps_big.tile([B, D2], F32, tag="big")
                for k in range(2):
                    nc.tensor.matmul(ps_u[:], hT["f"][:, k, :], W2T_sb[:, k, :],
                                     start=(k == 0), stop=(k == 1))
                u_sb = attn.tile([B, D2], F32, tag="u_sb", bufs=1)
                nc.vector.tensor_copy(out=u_sb[:], in_=ps_u[:])

                # uT_my[., j, i] = u[sel-batch i, 128j+.]  via  u_chunk.T @ Sel
                uT_my = attn.tile([128, 4, BL], F32, tag="uT_my")
                for j in range(4):
                    ps_t = ps_strip.tile([128, D2], F32, tag="strip")
                    nc.tensor.matmul(ps_t[:, :BL], u_sb[:, 128 * j:128 * (j + 1)],
                                     sel_sb[:], start=True, stop=True)
                    nc.vector.tensor_copy(out=uT_my[:, j, :], in_=ps_t[:, :BL])

                # aT = tanh(epT + uT_my), half-batch at a time (SBUF economy)
                # score strips land at partitions {0,32,64,96} of ps_sc[bh]
                ps_sc = [ps_strip.tile([128, S], F32, tag="strip", name=f"ps_sc{r_}") for r_ in range(2)]
                for r_ in range(2):
                    nc.vector.memset(ps_sc[r_][:], 0.0)
                for q in range(4):
                    aT = attn.tile([128, 4, BL // 4, S], F32, tag="aT", bufs=1,
                                   name="aT")
                    for j in range(4):
                        for i in range(BL // 4):
                            b = 2 * q + i
                            nc.vector.tensor_scalar_add(
                                out=aT[:, j, i, :], in0=epT_sb[:, j, b, :],
                                scalar1=uT_my[:, j, b:b + 1])
                    nc.scalar.activation(out=aT[:], in_=aT[:], func=AF.Tanh)
                    for i in range(BL // 4):
                        b = 2 * q + i
                        r, jc = divmod(b, 4)
                        for j in range(4):
                            nc.tensor.matmul(
                                ps_sc[r][32 * jc:32 * jc + 1, :],
                                VwT_sb[:, j:j + 1], aT[:, j, i, :],
                                start=(j == 0), stop=(j == 3),
                                tile_position=(0, 32 * jc))

                # drain strips (partition remap) via DMA, then dense softmax
                score = attn.tile([BL, S], F32, tag="score", bufs=1)
                for r in range(2):
                    ssb = attn.tile([128, D2], F32, tag="strip_sb", bufs=2,
                                    name="ssb")
                    nc.vector.tensor_copy(out=ssb[:, :S], in_=ps_sc[r][:])
                    nc.sync.dma_start(out=score[4 * r:4 * r + 4, :],
                                      in_=ssb[0:128:32, :S])
                negmax = attn.tile([BL, 1], F32, tag="negmax", bufs=1)
                nc.vector.tensor_reduce(out=negmax[:], in_=score[:],
                                        axis=mybir.AxisListType.X, op=ALU.max,
                                        negate=True)
                sumex = attn.tile([BL, 1], F32, tag="sumex", bufs=1)
                nc.scalar.activation(out=score[:], in_=score[:], func=AF.Exp,
                                     bias=negmax[:], scale=1.0,
                                     accum_out=sumex[:])
                rcpt = attn.tile([BL, 1], F32, tag="rcpt", bufs=1)
                nc.vector.reciprocal(out=rcpt[:], in_=sumex[:])
                nc.vector.tensor_scalar_mul(out=score[:], in0=score[:],
                                            scalar1=rcpt[:])

                # wT [s, b] via one PE transpose of [8, 128]
                ps_wT = ps_strip.tile([128, D2], F32, tag="strip")
                nc.tensor.transpose(ps_wT[:, :BL], score[:], ident_sb[:BL, :BL])
                wT = attn.tile([128, BL], F32, tag="wT")
                nc.vector.tensor_copy(out=wT[:], in_=ps_wT[:, :BL])

                # ctx strips: ctx[b] = w[b] @ enc[b] -> [1, D2] at partition 32*i
                ps_cx = [ps_strip.tile([128, D2], F32, tag="strip", name=f"ps_cx{r_}") for r_ in range(2)]
                for r_ in range(2):
                    nc.vector.memset(ps_cx[r_][:], 0.0)
                for b in range(BL):
                    r, i = divmod(b, 4)
                    nc.tensor.matmul(ps_cx[r][32 * i:32 * i + 1, :],
                                     wT[:, b:b + 1], enc_sb[:, b, :],
                                     start=True, stop=True,
                                     tile_position=(0, 32 * i))
                ctx = attn.tile([BL, D2], F32, tag="ctx", bufs=1)
                for r in range(2):
                    csb = attn.tile([128, D2], F32, tag="strip_sb", bufs=2,
                                    name="csb")
                    nc.vector.tensor_copy(out=csb[:], in_=ps_cx[r][:])
                    nc.sync.dma_start(out=ctx[4 * r:4 * r + 4, :],
                                      in_=csb[0:128:32, :])

                # ctxT chunks [128, BL] via PE transposes of dense [8, 128]
                ctxT = attn.tile([128, 4, BL], F32, tag="ctxT")
                for j in range(4):
                    ps_ct = ps_strip.tile([128, D2], F32, tag="strip")
                    nc.tensor.transpose(ps_ct[:, :BL],
                                        ctx[:, 128 * j:128 * (j + 1)],
                                        ident_sb[:BL, :BL])
                    nc.vector.tensor_copy(out=ctxT[:, j, :], in_=ps_ct[:, :BL])

                # ====== AllGather: [ctxT(512) ; cand(16)] x BL ======
                bnc_in = dram.tile([528, BL], F32, tag="bnc_in")
                bnc_out = dram.tile([CORES * 528, BL], F32, tag="bnc_out")
                nc.sync.dma_start(
                    out=bnc_in[0:512, :].rearrange("(j p) i -> p j i", p=128),
                    in_=ctxT[:])
                cand_src = prev_cand if prev_cand is not None else zero_cand
                nc.sync.dma_start(
                    out=bnc_in[512:528, :].rearrange("r (x j) -> (r x) j", j=2),
                    in_=cand_src[:])
                nc.gpsimd.collective_compute(
                    "AllGather", ALU.bypass,
                    replica_groups=[list(range(CORES))],
                    ins=[bnc_in[:].opt()],
                    outs=[bnc_out[:].opt()],
                )
                # ctx chunks -> xinT[:, 0:4, :]
                bo = bnc_out[:].rearrange("(r q) i -> r q i", q=528)
                for j in range(4):
                    nc.sync.dma_start(
                        out=xinT[:, j, :],
                        in_=bo[:, 128 * j:128 * (j + 1), :].rearrange(
                            "r p i -> p r i"))
                if prev_cand is not None:
                    candsAll = work.tile([B, CORES, 2], F32, tag="candsAll", bufs=1)
                    nc.sync.dma_start(
                        out=candsAll[:],
                        in_=bo[:, 512:528, :].rearrange(
                            "r q (x j) -> (q x) r j", j=2))
                    # global argmax among the 8 candidates
                    gmax = work.tile([B, 1], F32, tag="gmax")
                    nc.vector.tensor_reduce(out=gmax[:], in_=candsAll[:, :, 0],
                                            axis=mybir.AxisListType.X, op=ALU.max)
                    eq = work.tile([B, CORES], F32, tag="eq")
                    nc.vector.tensor_scalar(out=eq[:], in0=candsAll[:, :, 0],
                                            scalar1=gmax[:], scalar2=None,
                                            op0=ALU.is_equal)
                    prod = work.tile([B, CORES], F32, tag="prod")
                    nc.vector.tensor_mul(out=prod[:], in0=eq[:],
                                         in1=candsAll[:, :, 1])
                    widx_f = work.tile([B, 1], F32, tag="widx_f")
                    nc.vector.tensor_reduce(out=widx_f[:], in_=prod[:],
                                            axis=mybir.AxisListType.X, op=ALU.add)
                    widx_u = work.tile([B, 1], U32, tag="widx_u")
                    nc.vector.tensor_copy(out=widx_u[:], in_=widx_f[:])
                    emb_g = work.tile([B, H], F32, tag="emb_g", bufs=1)
                    nc.gpsimd.indirect_dma_start(
                        out=emb_g[:], out_offset=None,
                        in_=i_emb[:, :],
                        in_offset=bass.IndirectOffsetOnAxis(ap=widx_u[:], axis=0),
                        bounds_check=CORES * VSH - 1, oob_is_err=False)
                    for j in range(2):
                        ps_e = ps_strip.tile([128, D2], F32, tag="strip")
                        nc.tensor.transpose(ps_e[:, :B],
                                            emb_g[:, 128 * j:128 * (j + 1)],
                                            ident_sb[:B, :B])
                        nc.vector.tensor_copy(out=xinT[:, 4 + j, :],
                                              in_=ps_e[:, :B])

                # ====== LSTM (both dirs, full batch) ======
                tT = {}
                for d in ("f", "b"):
                    tT[d] = work.tile([128, 8, B], F32, tag=f"tT_{d}", name=f"tT_{d}", bufs=1)
                    for m in range(8):
                        ps_g = ps_lstm.tile([128, B], F32, tag="lstm")
                        msl = slice(128 * m, 128 * (m + 1))
                        # hT chunks first (ready earliest), then ctx, then emb
                        nc.tensor.matmul(ps_g[:], WhhT[d][:, 0, msl], hT[d][:, 0, :],
                                         start=True, stop=False)
                        nc.tensor.matmul(ps_g[:], WhhT[d][:, 1, msl], hT[d][:, 1, :],
                                         start=False, stop=False)
                        for k in range(6):
                            nc.tensor.matmul(ps_g[:], WihT[d][:, k, msl],
                                             xinT[:, k, :],
                                             start=False, stop=(k == 5))
                        nc.scalar.activation(out=tT[d][:, m, :], in_=ps_g[:],
                                             func=AF.Tanh,
                                             bias=bsc[d][:, m:m + 1], scale=1.0)
                for d in ("f", "b"):
                    ti = tT[d][:, 0:2, :]
                    tf = tT[d][:, 2:4, :]
                    tg = tT[d][:, 4:6, :]
                    to = tT[d][:, 6:8, :]
                    a1 = work.tile([128, 2, B], F32, tag="ga", name="ga1", bufs=2)
                    nc.vector.tensor_scalar_add(out=a1[:], in0=tf, scalar1=1.0)
                    m1 = work.tile([128, 2, B], F32, tag="gm", name="gm1", bufs=3)
                    nc.vector.tensor_mul(out=m1[:], in0=a1[:], in1=cT[d][:])
                    a2 = work.tile([128, 2, B], F32, tag="ga", name="ga2", bufs=2)
                    nc.vector.tensor_scalar_add(out=a2[:], in0=ti, scalar1=1.0)
                    m2 = work.tile([128, 2, B], F32, tag="gm", name="gm2", bufs=3)
                    nc.vector.tensor_mul(out=m2[:], in0=a2[:], in1=tg)
                    s12 = work.tile([128, 2, B], F32, tag="gs12", bufs=1)
                    nc.vector.tensor_add(out=s12[:], in0=m1[:], in1=m2[:])
                    nc.vector.tensor_scalar_mul(out=cT[d][:], in0=s12[:], scalar1=0.5)
                    tnc = work.tile([128, 2, B], F32, tag="gtnc", bufs=1)
                    nc.scalar.activation(out=tnc[:], in_=cT[d][:], func=AF.Tanh)
                    a3 = work.tile([128, 2, B], F32, tag="ga", name="ga3", bufs=2)
                    nc.vector.tensor_scalar_add(out=a3[:], in0=to, scalar1=1.0)
                    m3 = work.tile([128, 2, B], F32, tag="gm", name="gm3", bufs=3)
                    nc.vector.tensor_mul(out=m3[:], in0=a3[:], in1=tnc[:])
                    nc.vector.tensor_scalar_mul(out=hT[d][:], in0=m3[:], scalar1=0.5)

                if proj_mode != "f32":
                    outT_hi = work.tile([128, 4, B], BF16, tag="outT_hi")
                    outT_lo = work.tile([128, 4, B], BF16, tag="outT_lo")
                    o_tmp = work.tile([128, 4, B], F32, tag="outT_tmp")
                    for k in range(4):
                        src_h = hT["f"][:, k, :] if k < 2 else hT["b"][:, k - 2, :]
                        nc.vector.tensor_copy(out=outT_hi[:, k, :], in_=src_h)
                        nc.vector.tensor_copy(out=o_tmp[:, k, :], in_=outT_hi[:, k, :])
                        nc.vector.tensor_tensor(out=o_tmp[:, k, :], in0=src_h,
                                                in1=o_tmp[:, k, :], op=ALU.subtract)
                        nc.vector.tensor_copy(out=outT_lo[:, k, :], in_=o_tmp[:, k, :])

                # ====== projection -> logits [B, VSH], ys, local argmax ======
                logits = work.tile([B, VSH], F32, tag="logits", bufs=1)
                for g in range(8):
                    gsl = slice(512 * g, 512 * (g + 1))
                    ps_p = ps_big.tile([B, 512], F32, tag="big")
                    if proj_mode == "f32":
                        for k in range(4):
                            lhs = hT["f"][:, k, :] if k < 2 else hT["b"][:, k - 2, :]
                            nc.tensor.matmul(ps_p[:], lhs, FwT_sb[:, k, gsl],
                                             start=(k == 0), stop=(k == 3))
                    else:
                        # bf16 hi/lo split of out (computed below once per step)
                        for k in range(4):
                            nc.tensor.matmul(ps_p[:], outT_hi[:, k, :],
                                             FwT_hi_sb[:, k, gsl],
                                             start=(k == 0), stop=False)
                        for k in range(4):
                            nc.tensor.matmul(ps_p[:], outT_lo[:, k, :],
                                             FwT_hi_sb[:, k, gsl],
                                             start=False, stop=False)
                        for k in range(4):
                            nc.tensor.matmul(ps_p[:], outT_hi[:, k, :],
                                             FwT_lo_sb[:, k, gsl],
                                             start=False, stop=(k == 3))
                    nc.vector.tensor_copy(out=logits[:, gsl], in_=ps_p[:])
                if fb_nonzero:
                    nc.vector.tensor_add(out=logits[:], in0=logits[:], in1=FbB_sb[:])
                nc.sync.dma_start(out=o_ys[t], in_=logits[:])

                if t < T - 1:
                    mx8 = work.tile([B, 8], F32, tag="mx8")
                    nc.vector.max(mx8[:], logits[:])
                    idx8 = work.tile([B, 8], U32, tag="idx8")
                    nc.vector.max_index(idx8[:], mx8[:], logits[:])
                    idx_f = work.tile([B, 1], F32, tag="idx_f")
                    nc.vector.tensor_copy(out=idx_f[:], in_=idx8[:, 0:1])
                    cand = work.tile([B, 2], F32, tag="cand")
                    nc.vector.tensor_copy(out=cand[:, 0:1], in_=mx8[:, 0:1])
                    nc.vector.tensor_scalar(out=cand[:, 1:2], in0=idx_f[:],
                                            scalar1=vbase_sb[:], scalar2=None,
                                            op0=ALU.add)
                    prev_cand = cand

            # final state out
            for di, d in enumerate(("f", "b")):
                nc.sync.dma_start(
                    out=o_h[di].rearrange("(k p) b -> p k b", p=128), in_=hT[d][:])
                nc.sync.dma_start(
                    out=o_c[di].rearrange("(k p) b -> p k b", p=128), in_=cT[d][:])

    nc.compile()
    return nc


def host_prep(inputs, proj_mode: str):
    """Build the 8 per-core in_maps from the full-size inputs."""
    enc = np.asarray(inputs["encoder_output"], np.float32)   # [B, S, 2H]
    h0 = np.asarray(inputs["h0"], np.float32)
    c0 = np.asarray(inputs["c0"], np.float32)
    Uw = np.asarray(inputs["Uw"], np.float32)
    Ub = np.asarray(inputs["Ub"], np.float32)
    Ww = np.asarray(inputs["Ww"], np.float32)
    Wb = np.asarray(inputs["Wb"], np.float32)
    Vw = np.asarray(inputs["Vw"], np.float32)
    Wih = {"f": np.asarray(inputs["Wih_f"], np.float32),
           "b": np.asarray(inputs["Wih_b"], np.float32)}
    Whh = {"f": np.asarray(inputs["Whh_f"], np.float32),
           "b": np.asarray(inputs["Whh_b"], np.float32)}
    bb = {"f": np.asarray(inputs["b_f"], np.float32),
          "b": np.asarray(inputs["b_b"], np.float32)}
    Fw = np.asarray(inputs["Fw"], np.float32)
    Fb = np.asarray(inputs["Fb"], np.float32)
    embt = np.asarray(inputs["emb_table"], np.float32)

    # step-constant attention key projection, with Wb and Ub folded in
    ep = enc @ Ww.T + Wb[None, None, :] + Ub[None, None, :]   # [B, S, 2H]
    W2T = (Uw[:, :H] + Uw[:, H:]).T.copy()                    # [256, 512]
    VwT = Vw[0].reshape(4, 128).T.copy()                      # [128, 4]

    gate_scale = np.concatenate([
        np.full(2 * H, 0.5, np.float32),      # i, f
        np.full(H, 1.0, np.float32),          # g
        np.full(H, 0.5, np.float32),          # o
    ])
    WihT = {d: (Wih[d] * gate_scale[:, None]).T.copy() for d in "fb"}
    WhhT = {d: (Whh[d] * gate_scale[:, None]).T.copy() for d in "fb"}
    bsc = {d: (bb[d] * gate_scale).reshape(8, 128).T.copy() for d in "fb"}

    emb_pad = np.zeros((CORES * VSH, H), np.float32)
    emb_pad[:V] = embt
    embT0 = np.tile(embt[0][:, None], (1, B)).reshape(128, 2, B, order="F")
    # careful: want embT0[p, k, b] = emb[0][128k + p]
    embT0 = np.zeros((128, 2, B), np.float32)
    for k in range(2):
        embT0[:, k, :] = embt[0][128 * k:128 * (k + 1)][:, None]

    h0T = np.ascontiguousarray(np.transpose(h0, (0, 2, 1)))   # [2, 256, 64]
    c0T = np.ascontiguousarray(np.transpose(c0, (0, 2, 1)))

    fb_nonzero = bool(np.any(Fb))

    in_maps = []
    for c in range(CORES):
        bs = slice(BL * c, BL * (c + 1))
        encT_c = np.ascontiguousarray(np.transpose(enc[bs], (1, 0, 2)))  # [S, BL, D2]
        ep_c = ep[bs]                                          # [BL, S, D2]
        # epT[p, j, b, s] = ep_c[b, s, 128j + p]
        epT_c = np.ascontiguousarray(
            np.transpose(ep_c.reshape(BL, S, 4, 128), (3, 2, 0, 1)))
        sel_c = np.zeros((B, BL), np.float32)
        sel_c[np.arange(BL * c, BL * (c + 1)), np.arange(BL)] = 1.0
        vbase_c = np.full((B, 1), VSH * c, np.float32)
        Fw_c = np.zeros((VSH, D2), np.float32)
        lo, hi = VSH * c, min(VSH * (c + 1), V)
        Fw_c[:hi - lo] = Fw[lo:hi]
        FwT_c = Fw_c.T.copy()                                  # [512, 4096]
        m = {
            "encT": encT_c, "epT": epT_c,
            "h0T": h0T, "c0T": c0T,
            "W2T": W2T, "VwT": VwT,
            "WihT_f": WihT["f"], "WihT_b": WihT["b"],
            "WhhT_f": WhhT["f"], "WhhT_b": WhhT["b"],
            "bsc_f": bsc["f"], "bsc_b": bsc["b"],
            "emb": emb_pad, "embT0": embT0,
            "sel": sel_c, "vbase": vbase_c,
            "ident": np.eye(128, dtype=np.float32),
        }
        if proj_mode == "f32":
            m["FwT"] = FwT_c
        else:
            import ml_dtypes
            hi_ = FwT_c.astype(ml_dtypes.bfloat16)
            lo_ = (FwT_c - hi_.astype(np.float32)).astype(ml_dtypes.bfloat16)
            m["FwT_hi"] = hi_
            m["FwT_lo"] = lo_
        if fb_nonzero:
            Fb_c = np.full(VSH, -1e30, np.float32)
            Fb_c[:hi - lo] = Fb[lo:hi]
            m["FbB"] = np.tile(Fb_c[None, :], (B, 1))
        in_maps.append(m)
    return in_maps, fb_nonzero


def assemble(results, T):
    ys = np.concatenate([results[c]["ys_out"] for c in range(CORES)], axis=2)
    ys = np.ascontiguousarray(np.transpose(ys, (1, 0, 2))[:, :, :V])
    hT = np.ascontiguousarray(np.transpose(results[0]["hT_out"], (0, 2, 1)))
    cT = np.ascontiguousarray(np.transpose(results[0]["cT_out"], (0, 2, 1)))
    return ys, hT, cT


_BUILD_CACHE = {}


def run(inputs, proj_mode="f32", use_sim=False, check_with_hw=False):
    T = int(inputs["max_len"])
    in_maps, fb_nonzero = host_prep(inputs, proj_mode)
    key = (T, proj_mode, fb_nonzero)
    if key not in _BUILD_CACHE:
        _BUILD_CACHE[key] = build(T, proj_mode, fb_nonzero)
    nc = _BUILD_CACHE[key]
    if use_sim:
        from concourse.bass_interp import MultiCoreSim
        sim = MultiCoreSim(nc, CORES)
        for c in range(CORES):
            for k, v in in_maps[c].items():
                sim.cores[c].tensor(k)[:] = v
        sim.simulate(check_with_hw=check_with_hw)
        shapes = {"ys_out": (T, B, VSH), "hT_out": (2, H, B), "cT_out": (2, H, B)}
        results = [
            {k: np.array(sim.cores[c].mem_tensor(k)).reshape(shapes[k])
             for k in ("ys_out", "hT_out", "cT_out")}
            for c in range(CORES)
        ]
    else:
        res = bass_utils.run_bass_kernel_spmd(
            nc, in_maps, core_ids=list(range(CORES)))
        results = res.results
    return assemble(results, T)


def kernel(**inputs):
    return run(inputs, proj_mode="f32")
